# revision 15
# baseline (speedup 1.0000x reference)
"""CrossMoCo loss kernel for 8 Trainium2 NeuronCores.

Strategy (see sharding hint): shard the memory bank (M=65536 rows) across the
8 cores, 8192 rows each; q / labels are replicated.  Each core computes, for
its shard:
  - S1[b]    = sum_m exp(cos(q_b, p_m)/T)                (row exp-sums)
  - G[b, c]  = sum_d qn[b,d] * Z[c,d] / T   with  Z[c] = sum_{pl[m]==c} pn[m]
               (class-aggregated sum of targets*logits: S2[b] = G[b, labels[b]])
  - cnt[c]   = #{m : pl[m]==c}
plus the small [B,B] in-batch ("src") block terms (identical on every core).
The host sums the per-core partials and finishes the loss on [512]-vectors.

Only the exp-sum requires the full [B, M] logits; everything else collapses
through the C=10 label classes, so the kernel is one big bf16 matmul
(qn^T @ pn^T tiles), a fused exp+row-sum on the scalar engine, and a chain of
tiny class matmuls.
"""

import os
import sys

import numpy as np

for _p in ("/opt/trn_rl_repo", "/root/.axon_site/_ro/trn_rl_repo"):
    if os.path.isdir(_p) and _p not in sys.path:
        sys.path.append(_p)

import concourse.bass as bass
import concourse.tile as tile
from concourse import mybir
from concourse.bass_utils import run_bass_kernel_spmd
from concourse.masks import make_identity

F32 = mybir.dt.float32
BF16 = mybir.dt.bfloat16
I32 = mybir.dt.int32
AX = mybir.AxisListType
OP = mybir.AluOpType
AF = mybir.ActivationFunctionType

B = 512          # batch
D = 256          # feature dim
M = 65536        # memory rows
C = 10           # classes
N_CORES = 8
M_SH = M // N_CORES      # 8192 memory rows per core
TEMP = 0.07
INV_T = 1.0 / TEMP

P = 128          # partitions
NB = B // P      # 4 b-tiles
ND = D // P      # 2 d-halves
M_SC = 1024      # memory rows per superchunk
KT = M_SC // P   # 8 m-tiles per superchunk
SC = M_SH // M_SC  # 8 superchunks per core
NN = M_SC // 512   # 512-col matmul chunks per superchunk

QUAKE_MAGIC = 0x5F3759DF


def split_multi_waits(nc, max_waits=1):
    """Split multi-wait instructions into single-wait Drain preludes.

    The walrus build in this container accepts only one sync-wait per
    instruction, while Tile attaches several (notably on the kernel-tail
    Drain).  A preceding Drain on the same engine carrying one wait each is
    semantically equivalent (the engine stalls until every wait clears).
    """
    n_split = 0
    for bb in nc.main_func.blocks:
        insts = list(bb.instructions)
        out = []
        changed = False
        for ins in insts:
            si = ins.sync_info
            waits = list(si.on_wait) if si is not None and si.on_wait else []
            if len(waits) > max_waits:
                changed = True
                extra, keep = waits[:-max_waits], waits[-max_waits:]
                for i, w in enumerate(extra):
                    d = mybir.InstDrain(
                        name=f"{ins.name}-sw{i}",
                        opcode="Drain",
                        engine=ins.engine,
                        is_reset_sema=False,
                        sync_info=mybir.SyncInfo(on_wait=[w], on_update=[]),
                    )
                    d.debug = ins.debug
                    out.append(d)
                    n_split += 1
                ins.sync_info = mybir.SyncInfo(
                    on_wait=keep, on_update=list(si.on_update)
                )
            out.append(ins)
        if changed:
            bb.instructions = out
    return n_split


def _emit_rsqrt(nc, pool, ss, out, n):
    """out[:, :n] (f32) = 1/sqrt(ss[:, :n]) via quake seed + 3 Newton steps.

    Runs entirely on the vector engine so the scalar engine's table RAM stays
    on the Exp set for the whole kernel.
    """
    t_i = pool.tile([P, n], I32, tag="rsq_i")
    r_i = pool.tile([P, n], I32, tag="rsq_r")
    nc.vector.tensor_scalar(
        out=t_i, in0=ss.bitcast(I32), scalar1=1, scalar2=None,
        op0=OP.arith_shift_right,
    )
    # r_i = magic - (ss_i >> 1)  ==  t_i * -1 + magic
    nc.vector.tensor_scalar(
        out=r_i, in0=t_i, scalar1=-1, scalar2=QUAKE_MAGIC,
        op0=OP.mult, op1=OP.add,
    )
    r = r_i.bitcast(F32)
    for it in range(3):
        a = pool.tile([P, n], F32, tag="rsq_a")
        h = pool.tile([P, n], F32, tag="rsq_h")
        dst = out if it == 2 else pool.tile([P, n], F32, tag="rsq_n")
        nc.vector.tensor_mul(a, r, r)          # r^2
        nc.vector.tensor_mul(a, a, ss)         # ss * r^2
        nc.vector.tensor_scalar(               # 1.5 - 0.5*ss*r^2
            out=h, in0=a, scalar1=-0.5, scalar2=1.5, op0=OP.mult, op1=OP.add,
        )
        nc.vector.tensor_mul(dst, r, h)
        r = dst


def build_program(split_waits=True):
    nc = bass.Bass()

    pm = nc.dram_tensor("pm", [M_SH, D], F32, kind="ExternalInput")
    pl = nc.dram_tensor("pl", [M_SH, 2], I32, kind="ExternalInput")
    qd = nc.dram_tensor("qd", [B, D], F32, kind="ExternalInput")
    lab = nc.dram_tensor("lab", [B, 2], I32, kind="ExternalInput")

    # Per-core partials; host reorders [128, NB] -> [512].
    o_s1 = nc.dram_tensor("o_s1", [P, NB], F32, kind="ExternalOutput")
    o_g = nc.dram_tensor("o_g", [P, NB * C], F32, kind="ExternalOutput")
    o_cnt = nc.dram_tensor("o_cnt", [C, 1], F32, kind="ExternalOutput")
    o_srcexp = nc.dram_tensor("o_srcexp", [P, NB], F32, kind="ExternalOutput")
    o_diag = nc.dram_tensor("o_diag", [P, NB], F32, kind="ExternalOutput")
    o_s2s = nc.dram_tensor("o_s2s", [P, NB], F32, kind="ExternalOutput")
    o_n1s = nc.dram_tensor("o_n1s", [P, NB], F32, kind="ExternalOutput")

    pm_r = pm[:].rearrange("(s k p) d -> s p k d", p=P, k=KT)
    pl_r = pl[:].rearrange("(s k p) j -> s p k j", p=P, k=KT)
    q_r = qd[:].rearrange("(c p) d -> p c d", p=P)

    with tile.TileContext(nc) as tc:
        with (
            tc.tile_pool(name="const", bufs=1) as const,
            tc.tile_pool(name="outs", bufs=1) as outs,
            tc.tile_pool(name="pmp", bufs=2) as pmp,
            tc.tile_pool(name="plp", bufs=2) as plp,
            tc.tile_pool(name="pnp", bufs=2) as pnp,
            tc.tile_pool(name="pntp", bufs=4) as pntp,
            tc.tile_pool(name="lpp", bufs=3) as lpp,
            tc.tile_pool(name="scr", bufs=4) as scr,
            tc.tile_pool(name="expp", bufs=2) as expp,
            tc.tile_pool(name="lgp", bufs=2, space="PSUM") as lgp,
            tc.tile_pool(name="trp", bufs=2, space="PSUM") as trp,
            tc.tile_pool(name="zcp", bufs=1, space="PSUM") as zcp,
            tc.tile_pool(name="gp", bufs=1, space="PSUM") as gp,
        ):
            # ---------------- constants ----------------
            ident = const.tile([P, P], BF16)
            make_identity(nc, ident)
            iota10_i = const.tile([P, C], I32)
            nc.gpsimd.iota(iota10_i, pattern=[[1, C]], base=0, channel_multiplier=0)
            iota10 = const.tile([P, C], F32)
            nc.vector.tensor_copy(out=iota10, in_=iota10_i)
            iota10c_i = const.tile([C, 1], I32)
            nc.gpsimd.iota(iota10c_i, pattern=[[0, 1]], base=0, channel_multiplier=1)
            iota10c = const.tile([C, 1], F32)
            nc.vector.tensor_copy(out=iota10c, in_=iota10c_i)

            # ---------------- q: load, normalize, transpose ----------------
            q_sb = const.tile([P, NB, D], F32)
            nc.sync.dma_start(out=q_sb, in_=q_r)
            lab_bc_i = const.tile([C, B], I32)
            lab_bc_ap = bass.AP(tensor=lab[:].tensor, offset=0, ap=[[0, C], [2, B]])
            nc.gpsimd.dma_start(out=lab_bc_i, in_=lab_bc_ap)
            lab_bc = const.tile([C, B], F32)
            nc.vector.tensor_copy(out=lab_bc, in_=lab_bc_i)

            ss_q = const.tile([P, NB], F32)
            for c in range(NB):
                sq_s = scr.tile([P, D], F32, tag="sq")
                nc.vector.tensor_mul(sq_s, q_sb[:, c, :], q_sb[:, c, :])
                nc.vector.reduce_sum(out=ss_q[:, c : c + 1], in_=sq_s, axis=AX.X)
            rq = const.tile([P, NB], F32)
            _emit_rsqrt(nc, scr, ss_q, rq, NB)

            qn_bf = const.tile([P, NB, D], BF16)
            for c in range(NB):
                nc.vector.tensor_scalar_mul(
                    out=qn_bf[:, c, :], in0=q_sb[:, c, :], scalar1=rq[:, c : c + 1]
                )
            qnT = const.tile([P, ND, B], BF16)
            for d in range(ND):
                tq = trp.tile([P, B], BF16, tag="tr")
                for c in range(NB):
                    nc.tensor.transpose(
                        tq[:, c * P : (c + 1) * P],
                        qn_bf[:, c, d * P : (d + 1) * P],
                        ident,
                    )
                nc.vector.tensor_copy(out=qnT[:, d, :], in_=tq)

            # LqT[c, j] = (labels[j] == c), bf16 one-hot (class-major)
            lqT = const.tile([C, B], BF16)
            nc.vector.tensor_scalar(
                out=lqT, in0=lab_bc, scalar1=iota10c, scalar2=None, op0=OP.is_equal
            )

            # ---------------- src (in-batch) block ----------------
            srcexp_sb = outs.tile([P, NB], F32)
            diag_sb = outs.tile([P, NB], F32)
            s2s_sb = outs.tile([P, NB], F32)
            n1s_sb = outs.tile([P, NB], F32)
            smax = const.tile([P, NB], F32)
            for b in range(NB):
                sp = lgp.tile([P, B], F32, tag="lg")
                for d in range(ND):
                    nc.tensor.matmul(
                        sp, qnT[:, d, b * P : (b + 1) * P], qnT[:, d, :],
                        start=(d == 0), stop=(d == ND - 1),
                    )
                nc.vector.reduce_max(out=smax[:, b : b + 1], in_=sp, axis=AX.X)
                srclog = scr.tile([P, B], F32, tag="srclog")
                nc.vector.tensor_scalar(
                    out=srclog, in0=sp, scalar1=smax[:, b : b + 1], scalar2=None,
                    op0=OP.subtract,
                )
                es = expp.tile([P, M_SC], F32, tag="exp")
                nc.scalar.activation(
                    out=es[:, :B], in_=srclog, func=AF.Exp, scale=INV_T,
                    accum_out=srcexp_sb[:, b : b + 1],
                )
                dsel = scr.tile([P, B], F32, tag="dsel")
                nc.gpsimd.affine_select(
                    out=dsel, in_=srclog, compare_op=OP.is_equal, fill=0.0,
                    base=-(b * P), pattern=[[1, B]], channel_multiplier=-1,
                )
                nc.vector.reduce_sum(out=diag_sb[:, b : b + 1], in_=dsel, axis=AX.X)
                tp = lgp.tile([P, B], F32, tag="lg")
                nc.tensor.matmul(
                    tp, lqT[:, b * P : (b + 1) * P], lqT[:, :], start=True, stop=True
                )
                ttr_o = scr.tile([P, B], F32, tag="ttro")
                nc.vector.tensor_mul(ttr_o, tp, srclog)
                # s2s is in cos units here; host combine multiplies by 1/T
                nc.vector.reduce_sum(out=s2s_sb[:, b : b + 1], in_=ttr_o, axis=AX.X)
                nc.vector.reduce_sum(out=n1s_sb[:, b : b + 1], in_=tp, axis=AX.X)

            # ---------------- main loop over memory superchunks ----------------
            s1_cols = outs.tile([P, NB, SC], F32)
            zc = zcp.tile([C, D + 1], F32)  # [:, :D] = Z (class sums), [:, D] = cnt
            for sc in range(SC):
                pm_t = pmp.tile([P, KT, D], F32)
                nc.sync.dma_start(out=pm_t, in_=pm_r[sc])
                pl_t = plp.tile([P, KT, 2], I32)
                nc.gpsimd.dma_start(out=pl_t, in_=pl_r[sc])
                pl_f = plp.tile([P, KT], F32, tag="plf")
                nc.vector.tensor_copy(out=pl_f, in_=pl_t[:, :, 0])

                ss = scr.tile([P, KT], F32, tag="ss")
                for k in range(KT):
                    sq_s = scr.tile([P, D], F32, tag="sq")
                    nc.vector.tensor_mul(sq_s, pm_t[:, k, :], pm_t[:, k, :])
                    nc.vector.reduce_sum(out=ss[:, k : k + 1], in_=sq_s, axis=AX.X)
                rp = scr.tile([P, KT], F32, tag="rp")
                _emit_rsqrt(nc, scr, ss, rp, KT)

                pn_t = pnp.tile([P, KT, D + 1], BF16)
                nc.vector.memset(pn_t[:, :, D : D + 1], 1.0)
                for k in range(KT):
                    nc.vector.tensor_scalar_mul(
                        out=pn_t[:, k, :D], in0=pm_t[:, k, :],
                        scalar1=rp[:, k : k + 1],
                    )
                    lp_t = lpp.tile([P, C], BF16, tag="lp")
                    nc.vector.tensor_scalar(
                        out=lp_t, in0=iota10, scalar1=pl_f[:, k : k + 1],
                        scalar2=None, op0=OP.is_equal,
                    )
                    nc.tensor.matmul(
                        zc, lp_t, pn_t[:, k, :],
                        start=(sc == 0 and k == 0),
                        stop=(sc == SC - 1 and k == KT - 1),
                        skip_group_check=True,
                    )
                pnT = []
                for d in range(ND):
                    tr_t = trp.tile([P, M_SC], BF16, tag="tr")
                    for k in range(KT):
                        nc.tensor.transpose(
                            tr_t[:, k * P : (k + 1) * P],
                            pn_t[:, k, d * P : (d + 1) * P],
                            ident,
                        )
                    pnT_d = pntp.tile([P, M_SC], BF16, tag="pnt")
                    nc.vector.tensor_copy(out=pnT_d, in_=tr_t)
                    pnT.append(pnT_d)

                for b in range(NB):
                    lg = lgp.tile([P, M_SC], F32, tag="lg")
                    for n in range(NN):
                        for d in range(ND):
                            nc.tensor.matmul(
                                lg[:, n * 512 : (n + 1) * 512],
                                qnT[:, d, b * P : (b + 1) * P],
                                pnT[d][:, n * 512 : (n + 1) * 512],
                                start=(d == 0), stop=(d == ND - 1),
                            )
                    es = expp.tile([P, M_SC], F32, tag="exp")
                    nc.scalar.activation(
                        out=es, in_=lg, func=AF.Exp, scale=INV_T,
                        accum_out=s1_cols[:, b, sc : sc + 1],
                    )

            # ---------------- epilogue ----------------
            z_sb = const.tile([C, D], BF16)
            nc.vector.tensor_copy(out=z_sb, in_=zc[:, :D])
            cnt_sb = const.tile([C, 1], F32)
            nc.vector.tensor_copy(out=cnt_sb, in_=zc[:, D : D + 1])
            ztT = const.tile([P, ND, C], BF16)
            for d in range(ND):
                zt_p = trp.tile([P, C], BF16, tag="tr")
                nc.tensor.transpose(
                    zt_p, z_sb[0:C, d * P : (d + 1) * P], ident[0:C, 0:C]
                )
                nc.vector.tensor_copy(out=ztT[:, d, :], in_=zt_p)

            g_ps = gp.tile([P, NB * C], F32)
            for b in range(NB):
                for d in range(ND):
                    nc.tensor.matmul(
                        g_ps[:, b * C : (b + 1) * C],
                        qnT[:, d, b * P : (b + 1) * P],
                        ztT[:, d, :],
                        start=(d == 0), stop=(d == ND - 1),
                    )
            g_sb = outs.tile([P, NB * C], F32)
            nc.scalar.mul(out=g_sb, in_=g_ps, mul=INV_T)

            s1_sb = outs.tile([P, NB], F32)
            nc.vector.reduce_sum(out=s1_sb, in_=s1_cols, axis=AX.X)

            nc.sync.dma_start(out=o_s1[:], in_=s1_sb)
            nc.sync.dma_start(out=o_g[:], in_=g_sb)
            nc.sync.dma_start(out=o_cnt[:], in_=cnt_sb)
            nc.sync.dma_start(out=o_srcexp[:], in_=srcexp_sb)
            nc.sync.dma_start(out=o_diag[:], in_=diag_sb)
            nc.sync.dma_start(out=o_s2s[:], in_=s2s_sb)
            nc.sync.dma_start(out=o_n1s[:], in_=n1s_sb)

    if split_waits:
        split_multi_waits(nc)
    return nc


def make_in_maps(q, labels, pro_memory, pro_labels):
    q = np.ascontiguousarray(np.asarray(q), dtype=np.float32)
    pro_memory = np.ascontiguousarray(np.asarray(pro_memory), dtype=np.float32)
    labels_i = np.ascontiguousarray(np.asarray(labels), dtype=np.int64)
    pro_labels_i = np.ascontiguousarray(np.asarray(pro_labels), dtype=np.int64)
    lab32 = labels_i.view(np.int32).reshape(B, 2)
    pl32 = pro_labels_i.view(np.int32).reshape(M, 2)
    in_maps = []
    for c in range(N_CORES):
        in_maps.append(
            {
                "pm": np.ascontiguousarray(pro_memory[c * M_SH : (c + 1) * M_SH]),
                "pl": np.ascontiguousarray(pl32[c * M_SH : (c + 1) * M_SH]),
                "qd": q,
                "lab": lab32,
            }
        )
    return in_maps


def combine(results, labels):
    """Host-side unshard: sum per-core partials, finish the loss on [B] vectors."""
    labels_i = np.asarray(labels).astype(np.int64)

    def reorder(a):  # [128, NB] -> [512] with b = bt*128 + p
        return np.asarray(a, dtype=np.float64).T.reshape(B)

    s1 = np.zeros(B)
    g = np.zeros((B, C))
    cnt = np.zeros(C)
    for r in results:
        s1 += reorder(r["o_s1"])
        g += (
            np.asarray(r["o_g"], dtype=np.float64)
            .reshape(P, NB, C)
            .transpose(1, 0, 2)
            .reshape(B, C)
        )
        cnt += np.asarray(r["o_cnt"], dtype=np.float64).reshape(C)
    r0 = results[0]
    srcexp = reorder(r0["o_srcexp"])
    diag_cos = reorder(r0["o_diag"])
    s2s = reorder(r0["o_s2s"])
    n1s = reorder(r0["o_n1s"])

    diag_logit = diag_cos * INV_T
    denom = s1 + srcexp - np.exp(diag_logit)
    npos = cnt[labels_i] + n1s - 1.0
    s2 = g[np.arange(B), labels_i] + s2s * INV_T - diag_logit
    mean_log_prob_pos = (s2 - npos * np.log(denom)) / npos
    return np.float32(-np.mean(mean_log_prob_pos))


_nc_cache = {}


def kernel(q, labels, pro_memory, pro_labels):
    assert np.asarray(q).shape == (B, D)
    assert np.asarray(pro_memory).shape == (M, D)
    if "nc" not in _nc_cache:
        _nc_cache["nc"] = build_program()
    nc = _nc_cache["nc"]
    in_maps = make_in_maps(q, labels, pro_memory, pro_labels)
    res = run_bass_kernel_spmd(nc, in_maps, list(range(N_CORES))).results
    return combine(res, labels)


if __name__ == "__main__":
    rng = np.random.default_rng(0)
    q = rng.standard_normal((B, D)).astype(np.float32)
    labels = rng.integers(0, C, B).astype(np.int64)
    pm = rng.standard_normal((M, D)).astype(np.float32)
    pls = rng.integers(0, C, M).astype(np.int64)
    out = kernel(q, labels, pm, pls)
    print("kernel out:", out)


# revision 29
# speedup vs baseline: 1.2042x; 1.2042x over previous
"""CrossMoCo loss kernel for 8 Trainium2 NeuronCores.

Strategy (see sharding hint): shard the memory bank (M=65536 rows) across the
8 cores, 8192 rows each; q / labels are replicated.  Each core computes, for
its shard:
  - S1[b]    = sum_m exp(cos(q_b, p_m)/T)                (row exp-sums)
  - G[b, c]  = sum_d qn[b,d] * Z[c,d] / T   with  Z[c] = sum_{pl[m]==c} pn[m]
               (class-aggregated sum of targets*logits: S2[b] = G[b, labels[b]])
  - cnt[c]   = #{m : pl[m]==c}
plus the small [B,B] in-batch ("src") block terms (identical on every core).
The host sums the per-core partials and finishes the loss on [512]-vectors.

Only the exp-sum requires the full [B, M] logits; everything else collapses
through the C=10 label classes, so the kernel is one big bf16 matmul
(qn^T @ pn^T tiles), a fused exp+row-sum on the scalar engine, and a chain of
tiny class matmuls.
"""

import os
import sys

import numpy as np

for _p in ("/opt/trn_rl_repo", "/root/.axon_site/_ro/trn_rl_repo"):
    if os.path.isdir(_p) and _p not in sys.path:
        sys.path.append(_p)

import concourse.bass as bass
import concourse.tile as tile
from concourse import mybir
from concourse.bass_utils import run_bass_kernel_spmd
from concourse.masks import make_identity

F32 = mybir.dt.float32
BF16 = mybir.dt.bfloat16
I32 = mybir.dt.int32
AX = mybir.AxisListType
OP = mybir.AluOpType
AF = mybir.ActivationFunctionType

B = 512          # batch
D = 256          # feature dim
M = 65536        # memory rows
C = 10           # classes
N_CORES = 8
M_SH = M // N_CORES      # 8192 memory rows per core
TEMP = 0.07
INV_T = 1.0 / TEMP

P = 128          # partitions
NB = B // P      # 4 b-tiles
ND = D // P      # 2 d-halves
M_SC = 1024      # memory rows per superchunk
KT = M_SC // P   # 8 m-tiles per superchunk
SC = M_SH // M_SC  # 8 superchunks per core
NN = M_SC // 512   # 512-col matmul chunks per superchunk

QUAKE_MAGIC = 0x5F3759DF


def split_multi_waits(nc, max_waits=1):
    """Split multi-wait instructions into single-wait Drain preludes.

    The walrus build in this container accepts only one sync-wait per
    instruction, while Tile attaches several (notably on the kernel-tail
    Drain).  A preceding Drain on the same engine carrying one wait each is
    semantically equivalent (the engine stalls until every wait clears).
    """
    n_split = 0
    for bb in nc.main_func.blocks:
        insts = list(bb.instructions)
        out = []
        changed = False
        for ins in insts:
            si = ins.sync_info
            waits = list(si.on_wait) if si is not None and si.on_wait else []
            if len(waits) > max_waits:
                changed = True
                extra, keep = waits[:-max_waits], waits[-max_waits:]
                for i, w in enumerate(extra):
                    d = mybir.InstDrain(
                        name=f"{ins.name}-sw{i}",
                        opcode="Drain",
                        engine=ins.engine,
                        is_reset_sema=False,
                        sync_info=mybir.SyncInfo(on_wait=[w], on_update=[]),
                    )
                    d.debug = ins.debug
                    out.append(d)
                    n_split += 1
                ins.sync_info = mybir.SyncInfo(
                    on_wait=keep, on_update=list(si.on_update)
                )
            out.append(ins)
        if changed:
            bb.instructions = out
    return n_split


def _emit_rsqrt(nc, pool, ss, out, n, eng=None, iters=3):
    """out[:, :n] (f32) = 1/sqrt(ss[:, :n]) via quake seed + Newton steps.

    Runs on `eng` (default vector) so the scalar engine's table RAM stays on
    the Exp set for the whole kernel.
    """
    e = eng if eng is not None else nc.vector
    t_i = pool.tile([P, n], I32, tag="rsq_i")
    r_i = pool.tile([P, n], I32, tag="rsq_r")
    e.tensor_scalar(
        out=t_i, in0=ss.bitcast(I32), scalar1=1, scalar2=None,
        op0=OP.arith_shift_right,
    )
    # r_i = magic - (ss_i >> 1)  ==  t_i * -1 + magic
    e.tensor_scalar(
        out=r_i, in0=t_i, scalar1=-1, scalar2=QUAKE_MAGIC,
        op0=OP.mult, op1=OP.add,
    )
    r = r_i.bitcast(F32)
    for it in range(iters):
        a = pool.tile([P, n], F32, tag="rsq_a")
        h = pool.tile([P, n], F32, tag="rsq_h")
        dst = out if it == iters - 1 else pool.tile([P, n], F32, tag="rsq_n")
        e.tensor_mul(a, r, r)          # r^2
        e.tensor_mul(a, a, ss)         # ss * r^2
        e.tensor_scalar(               # 1.5 - 0.5*ss*r^2
            out=h, in0=a, scalar1=-0.5, scalar2=1.5, op0=OP.mult, op1=OP.add,
        )
        e.tensor_mul(dst, r, h)
        r = dst


def _bcast_free(ap, count):
    """Append a stride-0 innermost free dim (broadcast) to an AP."""
    return bass.AP(tensor=ap.tensor, offset=ap.offset, ap=list(ap.ap) + [[0, count]])


def build_program(split_waits=True):
    nc = bass.Bass()

    pm = nc.dram_tensor("pm", [M_SH, D], F32, kind="ExternalInput")
    # pro_labels / labels arrive host-repacked to partition-major int32 pairs
    # so the label DMAs are contiguous per partition (64B runs, no 8B spam).
    pl = nc.dram_tensor("pl", [SC * P, KT * 2], I32, kind="ExternalInput")
    qd = nc.dram_tensor("qd", [B, D], F32, kind="ExternalInput")
    lab = nc.dram_tensor("lab", [P, NB * 2], I32, kind="ExternalInput")

    # Per-core partials; host reorders [128, NB] -> [512].
    o_s1 = nc.dram_tensor("o_s1", [P, NB], F32, kind="ExternalOutput")
    o_g = nc.dram_tensor("o_g", [P, NB * C], F32, kind="ExternalOutput")
    o_cnt = nc.dram_tensor("o_cnt", [C, 1], F32, kind="ExternalOutput")
    o_srcexp = nc.dram_tensor("o_srcexp", [P, NB], F32, kind="ExternalOutput")
    o_diag = nc.dram_tensor("o_diag", [P, NB], F32, kind="ExternalOutput")
    o_s2s = nc.dram_tensor("o_s2s", [P, NB], F32, kind="ExternalOutput")
    o_n1s = nc.dram_tensor("o_n1s", [P, NB], F32, kind="ExternalOutput")

    pm_r = pm[:].rearrange("(s k p) d -> s p k d", p=P, k=KT)
    pl_r2 = pl[:].rearrange("(s p) (k j) -> s p k j", p=P, k=KT)
    q_r = qd[:].rearrange("(c p) d -> p c d", p=P)

    with tile.TileContext(nc) as tc:
        with (
            tc.tile_pool(name="const", bufs=1) as const,
            tc.tile_pool(name="outs", bufs=1) as outs,
            tc.tile_pool(name="pmp", bufs=2) as pmp,
            tc.tile_pool(name="plp", bufs=2) as plp,
            tc.tile_pool(name="pnp", bufs=2) as pnp,
            tc.tile_pool(name="pntp", bufs=4) as pntp,
            tc.tile_pool(name="lpp", bufs=3) as lpp,
            tc.tile_pool(name="scr", bufs=4) as scr,
            tc.tile_pool(name="expp", bufs=2) as expp,
            tc.tile_pool(name="lgp", bufs=2, space="PSUM") as lgp,
            tc.tile_pool(name="trp", bufs=2, space="PSUM") as trp,
            tc.tile_pool(name="zcp", bufs=1, space="PSUM") as zcp,
            tc.tile_pool(name="gp", bufs=1, space="PSUM") as gp,
        ):
            # ---------------- constants ----------------
            ident = const.tile([P, P], BF16)
            make_identity(nc, ident)
            iota10_i = const.tile([P, C], I32)
            nc.gpsimd.iota(iota10_i, pattern=[[1, C]], base=0, channel_multiplier=0)
            iota10 = const.tile([P, C], F32)
            nc.vector.tensor_copy(out=iota10, in_=iota10_i)
            iota10c_i = const.tile([C, 1], I32)
            nc.gpsimd.iota(iota10c_i, pattern=[[0, 1]], base=0, channel_multiplier=1)
            iota10c = const.tile([C, 1], F32)
            nc.vector.tensor_copy(out=iota10c, in_=iota10c_i)

            # ---------------- q: load, normalize, transpose ----------------
            q_sb = const.tile([P, NB, D], F32)
            nc.sync.dma_start(out=q_sb, in_=q_r)
            # labels in b-order on one partition, then replicate to C
            # partitions with a K=1 ones-matmul (no partition-broadcast DMA).
            lab_1i = const.tile([1, NB, P], I32)
            lab_1_ap = bass.AP(
                tensor=lab[:].tensor, offset=0,
                ap=[[0, 1], [2, NB], [NB * 2, P]],
            )
            nc.sync.dma_start(out=lab_1i, in_=lab_1_ap)
            lab_1f = const.tile([1, B], F32)
            nc.vector.tensor_copy(
                out=lab_1f, in_=lab_1i[:, :, :].rearrange("o t p -> o (t p)")
            )
            ones1 = const.tile([1, C], F32)
            nc.vector.memset(ones1, 1.0)

            ss_q = const.tile([P, NB], F32)
            sq_s = scr.tile([P, NB, D], F32, tag="sq")
            nc.vector.tensor_mul(sq_s, q_sb, q_sb)
            nc.vector.reduce_sum(out=ss_q, in_=sq_s, axis=AX.X)
            rq = const.tile([P, NB], F32)
            _emit_rsqrt(nc, scr, ss_q, rq, NB)

            qn_bf = const.tile([P, NB, D], BF16)
            nc.vector.tensor_mul(qn_bf, q_sb, _bcast_free(rq[:, :], D))
            qnT = const.tile([P, ND, B], BF16)
            for d in range(ND):
                tq = trp.tile([P, B], BF16, tag="tr")
                for c in range(NB):
                    nc.tensor.transpose(
                        tq[:, c * P : (c + 1) * P],
                        qn_bf[:, c, d * P : (d + 1) * P],
                        ident,
                    )
                nc.vector.tensor_copy(out=qnT[:, d, :], in_=tq)

            # LqT[c, j] = (labels[j] == c), bf16 one-hot (class-major)
            lab_ps = lgp.tile([C, B], F32, tag="lg")
            nc.tensor.matmul(lab_ps, ones1, lab_1f, start=True, stop=True)
            lqT = const.tile([C, B], BF16)
            nc.vector.tensor_scalar(
                out=lqT, in0=lab_ps, scalar1=iota10c, scalar2=None, op0=OP.is_equal
            )

            # ---------------- src (in-batch) block ----------------
            srcexp_sb = outs.tile([P, NB], F32)
            diag_sb = outs.tile([P, NB], F32)
            s2s_sb = outs.tile([P, NB], F32)
            n1s_sb = outs.tile([P, NB], F32)
            smax = const.tile([P, NB], F32)
            for b in range(NB):
                sp = lgp.tile([P, B], F32, tag="lg")
                for d in range(ND):
                    nc.tensor.matmul(
                        sp, qnT[:, d, b * P : (b + 1) * P], qnT[:, d, :],
                        start=(d == 0), stop=(d == ND - 1),
                    )
                nc.vector.reduce_max(out=smax[:, b : b + 1], in_=sp, axis=AX.X)
                srclog = scr.tile([P, B], F32, tag="srclog")
                nc.vector.tensor_scalar(
                    out=srclog, in0=sp, scalar1=smax[:, b : b + 1], scalar2=None,
                    op0=OP.subtract,
                )
                es = expp.tile([P, M_SC], F32, tag="exp")
                nc.scalar.activation(
                    out=es[:, :B], in_=srclog, func=AF.Exp, scale=INV_T,
                    accum_out=srcexp_sb[:, b : b + 1],
                )
                dsel = scr.tile([P, B], F32, tag="dsel")
                nc.gpsimd.affine_select(
                    out=dsel, in_=srclog, compare_op=OP.is_equal, fill=0.0,
                    base=-(b * P), pattern=[[1, B]], channel_multiplier=-1,
                )
                nc.vector.reduce_sum(out=diag_sb[:, b : b + 1], in_=dsel, axis=AX.X)
                tp = lgp.tile([P, B], F32, tag="lg")
                nc.tensor.matmul(
                    tp, lqT[:, b * P : (b + 1) * P], lqT[:, :], start=True, stop=True
                )
                ttr_o = scr.tile([P, B], F32, tag="ttro")
                nc.vector.tensor_mul(ttr_o, tp, srclog)
                # s2s is in cos units here; host combine multiplies by 1/T
                nc.vector.reduce_sum(out=s2s_sb[:, b : b + 1], in_=ttr_o, axis=AX.X)
                nc.vector.reduce_sum(out=n1s_sb[:, b : b + 1], in_=tp, axis=AX.X)

            # ---------------- main loop over memory superchunks ----------------
            s1_cols = outs.tile([P, NB, SC], F32)
            zc = zcp.tile([C, D + 1], F32)  # [:, :D] = Z (class sums), [:, D] = cnt
            for sc in range(SC):
                pm_t = pmp.tile([P, KT, D], F32)
                nc.sync.dma_start(out=pm_t, in_=pm_r[sc])
                pl_t = plp.tile([P, KT, 2], I32)
                nc.sync.dma_start(out=pl_t, in_=pl_r2[sc])
                pl_f = plp.tile([P, KT], F32, tag="plf")
                nc.vector.tensor_copy(out=pl_f, in_=pl_t[:, :, 0])

                ss = scr.tile([P, KT], F32, tag="ss")
                sq_s = scr.tile([P, KT, D], F32, tag="sq")
                nc.vector.tensor_mul(sq_s, pm_t, pm_t)
                nc.vector.reduce_sum(out=ss, in_=sq_s, axis=AX.X)
                rp = scr.tile([P, KT], F32, tag="rp")
                _emit_rsqrt(nc, scr, ss, rp, KT)

                pn_t = pnp.tile([P, KT, D + 1], BF16)
                nc.vector.memset(pn_t[:, :, D : D + 1], 1.0)
                nc.vector.tensor_mul(
                    pn_t[:, :, :D], pm_t, _bcast_free(rp[:, :], D)
                )
                lp_all = lpp.tile([P, KT, C], BF16, tag="lp")
                iota_b = bass.AP(
                    tensor=iota10[:, :].tensor, offset=iota10[:, :].offset,
                    ap=[iota10[:, :].ap[0], [0, KT], [1, C]],
                )
                nc.vector.tensor_tensor(
                    out=lp_all, in0=iota_b, in1=_bcast_free(pl_f[:, :], C),
                    op=OP.is_equal,
                )
                for k in range(KT):
                    nc.tensor.matmul(
                        zc, lp_all[:, k, :], pn_t[:, k, :],
                        start=(sc == 0 and k == 0),
                        stop=(sc == SC - 1 and k == KT - 1),
                        skip_group_check=True,
                    )
                pnT = []
                for d in range(ND):
                    tr_t = trp.tile([P, M_SC], BF16, tag="tr")
                    for k in range(KT):
                        nc.tensor.transpose(
                            tr_t[:, k * P : (k + 1) * P],
                            pn_t[:, k, d * P : (d + 1) * P],
                            ident,
                        )
                    pnT_d = pntp.tile([P, M_SC], BF16, tag="pnt")
                    nc.vector.tensor_copy(out=pnT_d, in_=tr_t)
                    pnT.append(pnT_d)

                for b in range(NB):
                    lg = lgp.tile([P, M_SC], F32, tag="lg")
                    for n in range(NN):
                        for d in range(ND):
                            nc.tensor.matmul(
                                lg[:, n * 512 : (n + 1) * 512],
                                qnT[:, d, b * P : (b + 1) * P],
                                pnT[d][:, n * 512 : (n + 1) * 512],
                                start=(d == 0), stop=(d == ND - 1),
                            )
                    es = expp.tile([P, M_SC], F32, tag="exp")
                    nc.scalar.activation(
                        out=es, in_=lg, func=AF.Exp, scale=INV_T,
                        accum_out=s1_cols[:, b, sc : sc + 1],
                    )

            # ---------------- epilogue ----------------
            z_sb = const.tile([C, D], BF16)
            nc.vector.tensor_copy(out=z_sb, in_=zc[:, :D])
            cnt_sb = const.tile([C, 1], F32)
            nc.vector.tensor_copy(out=cnt_sb, in_=zc[:, D : D + 1])
            ztT = const.tile([P, ND, C], BF16)
            for d in range(ND):
                zt_p = trp.tile([P, C], BF16, tag="tr")
                nc.tensor.transpose(
                    zt_p, z_sb[0:C, d * P : (d + 1) * P], ident[0:C, 0:C]
                )
                nc.vector.tensor_copy(out=ztT[:, d, :], in_=zt_p)

            g_ps = gp.tile([P, NB * C], F32)
            for b in range(NB):
                for d in range(ND):
                    nc.tensor.matmul(
                        g_ps[:, b * C : (b + 1) * C],
                        qnT[:, d, b * P : (b + 1) * P],
                        ztT[:, d, :],
                        start=(d == 0), stop=(d == ND - 1),
                    )
            g_sb = outs.tile([P, NB * C], F32)
            nc.scalar.mul(out=g_sb, in_=g_ps, mul=INV_T)

            s1_sb = outs.tile([P, NB], F32)
            nc.vector.reduce_sum(out=s1_sb, in_=s1_cols, axis=AX.X)

            nc.sync.dma_start(out=o_s1[:], in_=s1_sb)
            nc.sync.dma_start(out=o_g[:], in_=g_sb)
            nc.sync.dma_start(out=o_cnt[:], in_=cnt_sb)
            nc.sync.dma_start(out=o_srcexp[:], in_=srcexp_sb)
            nc.sync.dma_start(out=o_diag[:], in_=diag_sb)
            nc.sync.dma_start(out=o_s2s[:], in_=s2s_sb)
            nc.sync.dma_start(out=o_n1s[:], in_=n1s_sb)

    if split_waits:
        split_multi_waits(nc)
    return nc


def make_in_maps(q, labels, pro_memory, pro_labels):
    q = np.ascontiguousarray(np.asarray(q), dtype=np.float32)
    pro_memory = np.ascontiguousarray(np.asarray(pro_memory), dtype=np.float32)
    labels_i = np.ascontiguousarray(np.asarray(labels), dtype=np.int64)
    pro_labels_i = np.ascontiguousarray(np.asarray(pro_labels), dtype=np.int64)
    lab32 = labels_i.view(np.int32).reshape(B, 2)
    pl32 = pro_labels_i.view(np.int32).reshape(M, 2)
    # partition-major repack so the label DMAs are contiguous per partition:
    # lab[p, bt*2+j] = labels[bt*128+p];  pl[sc*128+p, k*2+j] = pl[(sc*KT+k)*128+p]
    lab_pk = np.ascontiguousarray(
        lab32.reshape(NB, P, 2).transpose(1, 0, 2).reshape(P, NB * 2)
    )
    in_maps = []
    for c in range(N_CORES):
        plc = pl32[c * M_SH : (c + 1) * M_SH]
        pl_pk = np.ascontiguousarray(
            plc.reshape(SC, KT, P, 2).transpose(0, 2, 1, 3).reshape(SC * P, KT * 2)
        )
        in_maps.append(
            {
                "pm": np.ascontiguousarray(pro_memory[c * M_SH : (c + 1) * M_SH]),
                "pl": pl_pk,
                "qd": q,
                "lab": lab_pk,
            }
        )
    return in_maps


def combine(results, labels):
    """Host-side unshard: sum per-core partials, finish the loss on [B] vectors."""
    labels_i = np.asarray(labels).astype(np.int64)

    def reorder(a):  # [128, NB] -> [512] with b = bt*128 + p
        return np.asarray(a, dtype=np.float64).T.reshape(B)

    s1 = np.zeros(B)
    g = np.zeros((B, C))
    cnt = np.zeros(C)
    for r in results:
        s1 += reorder(r["o_s1"])
        g += (
            np.asarray(r["o_g"], dtype=np.float64)
            .reshape(P, NB, C)
            .transpose(1, 0, 2)
            .reshape(B, C)
        )
        cnt += np.asarray(r["o_cnt"], dtype=np.float64).reshape(C)
    r0 = results[0]
    srcexp = reorder(r0["o_srcexp"])
    diag_cos = reorder(r0["o_diag"])
    s2s = reorder(r0["o_s2s"])
    n1s = reorder(r0["o_n1s"])

    diag_logit = diag_cos * INV_T
    denom = s1 + srcexp - np.exp(diag_logit)
    npos = cnt[labels_i] + n1s - 1.0
    s2 = g[np.arange(B), labels_i] + s2s * INV_T - diag_logit
    mean_log_prob_pos = (s2 - npos * np.log(denom)) / npos
    return np.float32(-np.mean(mean_log_prob_pos))


_nc_cache = {}


def kernel(q, labels, pro_memory, pro_labels):
    assert np.asarray(q).shape == (B, D)
    assert np.asarray(pro_memory).shape == (M, D)
    if "nc" not in _nc_cache:
        _nc_cache["nc"] = build_program()
    nc = _nc_cache["nc"]
    in_maps = make_in_maps(q, labels, pro_memory, pro_labels)
    res = run_bass_kernel_spmd(nc, in_maps, list(range(N_CORES))).results
    return combine(res, labels)


if __name__ == "__main__":
    rng = np.random.default_rng(0)
    q = rng.standard_normal((B, D)).astype(np.float32)
    labels = rng.integers(0, C, B).astype(np.int64)
    pm = rng.standard_normal((M, D)).astype(np.float32)
    pls = rng.integers(0, C, M).astype(np.int64)
    out = kernel(q, labels, pm, pls)
    print("kernel out:", out)
